# revision 11
# baseline (speedup 1.0000x reference)
"""Trainium2 Bass kernel for llama-style GQA attention block.

Problem (hardcoded): x[1,2048,2048] f32, 32 q heads / 8 kv heads, head_dim 64,
RoPE (interleaved pairs), causal mask, out proj. 8-core tensor parallel across
heads: each core owns 4 q heads + 1 kv head, computes its slice end-to-end
including its wo row-block partial product; host sums the 8 partials.

All matmuls run as float32r (fp32 data, fast PE mode, ~1e-4 rel err).
Layout is "feature-on-partition" (transposed) throughout so every matmul
contracts over the partition dim with no on-chip transposes of activations:
  QT = wq^T x^T       (via lhsT=wq, rhs=xT)
  ST = K Q^T          (via lhsT=KT, rhs=QT)   -> softmax along partitions
  OT = [V|1]^T PT     (via lhsT=Vext, rhs=PT) -> row 64 = softmax denominator
  out = OT^T wo       (via lhsT=OT, rhs=wo)
RoPE even/odd pairs are separated at the wq/wk column-permutation level
(host-side), so on-chip RoPE is 6 full-width DVE ops per chunk.
"""

import numpy as np

import concourse.bass as bass
import concourse.bacc as bacc
import concourse.mybir as mybir
from concourse.tile import TileContext
from concourse.bass_utils import run_bass_kernel_spmd

DIM = 2048
SEQ = 2048
N_HEADS = 32
N_KV = 8
HD = 64
NCORES = 8
HPC = N_HEADS // NCORES      # 4 q heads per core
SC = 512                     # seq chunk (matmul free dim)
NSC = SEQ // SC              # 4
KTILE = 128
NKT = SEQ // KTILE           # 16
NDCH = DIM // 128            # 16 contraction chunks for projections
F32 = mybir.dt.float32
F32R = mybir.dt.float32r
NEG = -1.0e30

_CACHE = {}


def _build_nc():
    nc = bacc.Bacc("TRN2", debug=False, num_devices=NCORES)
    xT_p = nc.declare_dram_parameter("xT", [DIM, SEQ], F32R, isOutput=False)
    wq_p = nc.declare_dram_parameter("wq", [DIM, HPC * HD], F32R, isOutput=False)
    wkv_p = nc.declare_dram_parameter("wkv", [DIM, 2 * HD], F32R, isOutput=False)
    wo_p = nc.declare_dram_parameter("wo", [HPC * HD, DIM], F32R, isOutput=False)
    cs_p = nc.declare_dram_parameter("cs", [2 * HD, SEQ], F32, isOutput=False)
    pat_p = nc.declare_dram_parameter("pat", [4 * KTILE, SC], F32, isOutput=False)
    id_p = nc.declare_dram_parameter("ident", [HD, HD], F32, isOutput=False)
    out_p = nc.declare_dram_parameter("out", [SEQ, DIM], F32, isOutput=True)

    xT_r = xT_p.rearrange("(k p) s -> k p s", p=128)
    wq_r = wq_p.rearrange("(k p) m -> p k m", p=128)
    wkv_r = wkv_p.rearrange("(k p) m -> p k m", p=128)
    pat_r = pat_p.rearrange("(j p) q -> p j q", p=128)
    EXP = mybir.ActivationFunctionType.Exp

    with TileContext(nc) as tc:
        with (
            tc.tile_pool(name="res", bufs=1) as res,
            tc.tile_pool(name="sb", bufs=2) as sb,
            tc.tile_pool(name="psum", bufs=2, space="PSUM") as psum,
        ):
            # ---- resident loads ----
            wq_t = res.tile([128, NDCH, HPC * HD], F32R, tag="wq_t")
            nc.sync.dma_start(out=wq_t[:], in_=wq_r)
            wkv_t = res.tile([128, NDCH, 2 * HD], F32R, tag="wkv_t")
            nc.sync.dma_start(out=wkv_t[:], in_=wkv_r)
            wo0_t = res.tile([128, DIM], F32R, tag="wo0_t")
            nc.sync.dma_start(out=wo0_t[:], in_=wo_p[0:128, :])
            wo1_t = res.tile([128, DIM], F32R, tag="wo1_t")
            nc.sync.dma_start(out=wo1_t[:], in_=wo_p[128:256, :])
            cos64 = res.tile([HD, SEQ], F32, tag="cos64")
            nc.sync.dma_start(out=cos64[:], in_=cs_p[0:HD, :])
            sin64 = res.tile([HD, SEQ], F32, tag="sin64")
            nc.sync.dma_start(out=sin64[:], in_=cs_p[HD : 2 * HD, :])
            pat_t = res.tile([128, 4, SC], F32, tag="pat_t")
            nc.sync.dma_start(out=pat_t[:], in_=pat_r)
            ident = res.tile([HD, HD], F32, tag="ident")
            nc.sync.dma_start(out=ident[:], in_=id_p[:, :])

            # ---- resident intermediates ----
            QeA = res.tile([HD, SEQ], F32R, tag="QeA")   # heads 0,1 even-pair rot
            QoA = res.tile([HD, SEQ], F32R, tag="QoA")
            QeB = res.tile([HD, SEQ], F32R, tag="QeB")   # heads 2,3
            QoB = res.tile([HD, SEQ], F32R, tag="QoB")
            KrepE = res.tile([HD, SEQ], F32R, tag="KrepE")  # kv head, 2 copies
            KrepO = res.tile([HD, SEQ], F32R, tag="KrepO")
            VT_sb = res.tile([HD, SEQ], F32, tag="VT_sb")
            OTn0 = res.tile([128, SEQ], F32R, tag="OTn0")   # heads 0,1 norm out^T
            OTn1 = res.tile([128, SEQ], F32R, tag="OTn1")
            ones_col = res.tile([128, 1], F32, tag="ones_col")
            nc.vector.memset(ones_col[:], 1.0)
            ones_f32 = res.tile([1, HD], F32, tag="ones_f32")
            nc.vector.memset(ones_f32[:], 1.0)
            ones_row = res.tile([1, HD], F32R, tag="ones_row")
            nc.vector.tensor_copy(ones_row[:], ones_f32[:])
            vext = []
            for kt in range(NKT):
                vx = res.tile([128, HD + 1], F32R, tag=f"vx{kt}")
                nc.vector.tensor_copy(vx[:, HD : HD + 1], ones_col[:])
                vext.append(vx)

            # ---- phase 1: QKV projections + rope ----
            for sc in range(NSC):
                slc = slice(sc * SC, (sc + 1) * SC)
                qa_ps = psum.tile([128, SC], F32, tag="psA")
                qb_ps = psum.tile([128, SC], F32, tag="psB")
                kv_ps = psum.tile([128, SC], F32, tag="psC")
                for k in range(NDCH):
                    xt = sb.tile([128, SC], F32R, tag="xt", bufs=3)
                    nc.sync.dma_start(out=xt[:], in_=xT_r[k, :, slc])
                    st, sp = (k == 0), (k == NDCH - 1)
                    nc.tensor.matmul(qa_ps[:], (wq_t[:, k, 0:128]), (xt[:]), start=st, stop=sp)
                    nc.tensor.matmul(qb_ps[:], (wq_t[:, k, 128:256]), (xt[:]), start=st, stop=sp)
                    nc.tensor.matmul(kv_ps[:], (wkv_t[:, k, :]), (xt[:]), start=st, stop=sp)
                # rope Q, head groups A (rows 0:64 even / 64:128 odd) and B
                for grp, (qps, Qe, Qo) in enumerate([(qa_ps, QeA, QoA), (qb_ps, QeB, QoB)]):
                    t1 = sb.tile([HD, SC], F32, tag="t1", bufs=2)
                    t2 = sb.tile([HD, SC], F32, tag="t2", bufs=2)
                    nc.vector.tensor_mul(t1[:], qps[0:HD, :], cos64[:, slc])
                    nc.vector.tensor_mul(t2[:], qps[HD:128, :], sin64[:, slc])
                    nc.vector.tensor_sub(Qe[:, slc], t1[:], t2[:])
                    t3 = sb.tile([HD, SC], F32, tag="t3", bufs=2)
                    t4 = sb.tile([HD, SC], F32, tag="t4", bufs=2)
                    nc.vector.tensor_mul(t3[:], qps[HD:128, :], cos64[:, slc])
                    nc.vector.tensor_mul(t4[:], qps[0:HD, :], sin64[:, slc])
                    nc.vector.tensor_add(Qo[:, slc], t3[:], t4[:])
                # rope K (rows 0:32 even / 32:64 odd of kv_ps), then duplicate
                k1 = sb.tile([32, SC], F32, tag="k1", bufs=2)
                k2 = sb.tile([32, SC], F32, tag="k2", bufs=2)
                nc.vector.tensor_mul(k1[:], kv_ps[0:32, :], cos64[0:32, slc])
                nc.vector.tensor_mul(k2[:], kv_ps[32:64, :], sin64[0:32, slc])
                nc.vector.tensor_sub(KrepE[0:32, slc], k1[:], k2[:])
                k3 = sb.tile([32, SC], F32, tag="k3", bufs=2)
                k4 = sb.tile([32, SC], F32, tag="k4", bufs=2)
                nc.vector.tensor_mul(k3[:], kv_ps[32:64, :], cos64[0:32, slc])
                nc.vector.tensor_mul(k4[:], kv_ps[0:32, :], sin64[0:32, slc])
                nc.vector.tensor_add(KrepO[0:32, slc], k3[:], k4[:])
                nc.vector.tensor_copy(KrepE[32:64, slc], KrepE[0:32, slc])
                nc.vector.tensor_copy(KrepO[32:64, slc], KrepO[0:32, slc])
                # V passthrough
                nc.any.tensor_copy(VT_sb[:, slc], kv_ps[HD:128, :])

            # ---- phase 1.5: V transpose into [sk, hd | 1] tiles ----
            for kt in range(NKT):
                vt_ps = psum.tile([128, HD], F32, tag="psD")
                nc.tensor.transpose(
                    vt_ps[:], VT_sb[:, kt * 128 : (kt + 1) * 128], ident[:]
                )
                nc.any.tensor_copy(vext[kt][:, 0:HD], vt_ps[:])

            # ---- phase 2: attention ----
            for sc in range(NSC):
                slc = slice(sc * SC, (sc + 1) * SC)
                nkt_h = 4 * sc + 4
                for h in range(HPC):
                    g, hh = h // 2, h % 2
                    Qe = (QeA, QeB)[g]
                    Qo = (QoA, QoB)[g]
                    rows = slice(32 * hh, 32 * hh + 32)
                    o_ps = psum.tile([HD + 1, SC], F32, tag="psB")
                    for kt in range(nkt_h):
                        ksl = slice(kt * 128, (kt + 1) * 128)
                        st_ps = psum.tile([128, SC], F32, tag="psA")
                        nc.tensor.matmul(st_ps[:], (KrepE[rows, ksl]), (Qe[rows, slc]), start=True, stop=False)
                        nc.tensor.matmul(st_ps[:], (KrepO[rows, ksl]), (Qo[rows, slc]), start=False, stop=True)
                        j = kt - 4 * sc
                        if j >= 0:
                            nc.vector.tensor_add(st_ps[:], st_ps[:], pat_t[:, j, :])
                        ptile = sb.tile([128, SC], F32R, tag="pt", bufs=6)
                        nc.scalar.activation(ptile[:], st_ps[:], EXP, scale=0.125)
                        nc.tensor.matmul(o_ps[:], (vext[kt][:]), (ptile[:]), start=(kt == 0), stop=(kt == nkt_h - 1))
                    recip = sb.tile([1, SC], F32R, tag="recip", bufs=2)
                    with nc.allow_low_precision(reason="f32r is fp32-width for DVE"):
                        nc.vector.reciprocal(recip[:], o_ps[HD : HD + 1, :])
                    bc_ps = psum.tile([HD, SC], F32, tag="psD")
                    nc.tensor.matmul(bc_ps[:], ones_row[:], recip[:], start=True, stop=True)
                    ou_sb = sb.tile([HD, SC], F32, tag="ou", bufs=2)
                    nc.any.tensor_copy(ou_sb[:], o_ps[0:HD, :])
                    dst = (OTn0, OTn1)[g]
                    nc.vector.tensor_mul(dst[64 * hh : 64 * hh + 64, slc], ou_sb[:], bc_ps[:])

            # ---- phase 3: output projection ----
            for st in range(NKT):
                ssl = slice(st * 128, (st + 1) * 128)
                for dch in range(NSC):
                    dsl = slice(dch * SC, (dch + 1) * SC)
                    op_ps = psum.tile([128, SC], F32, tag="psC")
                    nc.tensor.matmul(op_ps[:], (OTn0[:, ssl]), (wo0_t[:, dsl]), start=True, stop=False)
                    nc.tensor.matmul(op_ps[:], (OTn1[:, ssl]), (wo1_t[:, dsl]), start=False, stop=True)
                    ot = sb.tile([128, SC], F32, tag="ot", bufs=4)
                    nc.any.tensor_copy(ot[:], op_ps[:])
                    nc.sync.dma_start(out=out_p[ssl, dsl], in_=ot[:])

    nc.compile()
    return nc


def _host_prep(x, freqs_cos, freqs_sin):
    """Shared (core-independent) host-side tensors."""
    xT = np.ascontiguousarray(np.asarray(x, np.float32)[0].T)          # [DIM, SEQ]
    cosT = np.ascontiguousarray(np.asarray(freqs_cos, np.float32).T)   # [32, SEQ]
    sinT = np.ascontiguousarray(np.asarray(freqs_sin, np.float32).T)
    cs = np.concatenate([np.tile(cosT, (2, 1)), np.tile(sinT, (2, 1))], 0)  # [128, SEQ]
    kk = np.arange(4 * KTILE)[:, None]
    qq = np.arange(SC)[None, :]
    pat = np.where(kk <= qq, 0.0, NEG).astype(np.float32)              # [512, 512]
    return xT, cs, pat


def _perm_eo(ncols_heads):
    """Column permutation. For head groups (1 or 2 heads per group):
    [h0 even, h1 even, h0 odd, h1 odd] — splits RoPE pairs into separate rows."""
    perm = []
    nheads = ncols_heads // HD
    groups = [(2 * g, 2 * g + 1) for g in range(nheads // 2)] if nheads > 1 else [(0,)]
    for grp in groups:
        for h in grp:
            perm += [h * HD + 2 * i for i in range(HD // 2)]
        for h in grp:
            perm += [h * HD + 2 * i + 1 for i in range(HD // 2)]
    return perm


def _is_causal(mask):
    m = np.asarray(mask)
    if m.shape != (SEQ, SEQ):
        return False
    tril = np.tril(np.ones((SEQ, SEQ), bool))
    return bool(np.all(m[tril] == 0.0) and np.all(np.isneginf(m[~tril])))


def _numpy_fallback(x, freqs_cos, freqs_sin, mask, wq, wk, wv, wo):
    x = np.asarray(x, np.float64)
    b, s, _ = x.shape
    xq = (x @ wq).reshape(b, s, N_HEADS, HD)
    xk = (x @ wk).reshape(b, s, N_KV, HD)
    xv = (x @ wv).reshape(b, s, N_KV, HD)

    def rope(t):
        t2 = t.reshape(*t.shape[:-1], HD // 2, 2)
        te, to = t2[..., 0], t2[..., 1]
        c = np.asarray(freqs_cos, np.float64)[None, :, None, :]
        sn = np.asarray(freqs_sin, np.float64)[None, :, None, :]
        oe = te * c - to * sn
        oo = te * sn + to * c
        return np.stack([oe, oo], -1).reshape(t.shape)

    xq, xk = rope(xq), rope(xk)
    xk = np.repeat(xk, N_HEADS // N_KV, axis=2)
    xv = np.repeat(xv, N_HEADS // N_KV, axis=2)
    sc_ = np.einsum("bqhd,bkhd->bhqk", xq, xk) / np.sqrt(HD)
    sc_ = sc_ + np.asarray(mask, np.float64)[None, None]
    m = sc_.max(-1, keepdims=True)
    p = np.exp(sc_ - m)
    p = p / p.sum(-1, keepdims=True)
    out = np.einsum("bhqk,bkhd->bqhd", p, xv).reshape(b, s, N_HEADS * HD)
    return (out @ wo).astype(np.float32)


def _make_in_maps(x, freqs_cos, freqs_sin, wq, wk, wv, wo):
    xT, cs, pat = _host_prep(x, freqs_cos, freqs_sin)
    wq = np.asarray(wq, np.float32)
    wk = np.asarray(wk, np.float32)
    wv = np.asarray(wv, np.float32)
    wo = np.asarray(wo, np.float32)
    permq = _perm_eo(HPC * HD)
    permk = _perm_eo(HD)
    in_maps = []
    for c in range(NCORES):
        wq_c = np.ascontiguousarray(wq[:, c * 256 : (c + 1) * 256][:, permq])
        wk_c = wk[:, c * HD : (c + 1) * HD][:, permk]
        wv_c = wv[:, c * HD : (c + 1) * HD]
        wkv_c = np.ascontiguousarray(np.concatenate([wk_c, wv_c], 1))
        wo_c = np.ascontiguousarray(wo[c * 256 : (c + 1) * 256, :])
        im = {"xT": xT, "wq": wq_c, "wkv": wkv_c, "wo": wo_c, "cs": cs, "pat": pat,
              "ident": np.eye(HD, dtype=np.float32)}
        expect = {"xT": (DIM, SEQ), "wq": (DIM, HPC * HD), "wkv": (DIM, 2 * HD),
                  "wo": (HPC * HD, DIM), "cs": (2 * HD, SEQ), "pat": (4 * KTILE, SC),
                  "ident": (HD, HD)}
        for k_, v_ in im.items():
            assert v_.shape == expect[k_], (k_, v_.shape, expect[k_])
        in_maps.append(im)
    return in_maps


def get_nc():
    if "nc" not in _CACHE:
        _CACHE["nc"] = _build_nc()
    return _CACHE["nc"]


def kernel(x, freqs_cos, freqs_sin, mask, wq, wk, wv, wo):
    if not _is_causal(mask):
        return _numpy_fallback(x, freqs_cos, freqs_sin, mask, wq, wk, wv, wo)
    nc = get_nc()
    in_maps = _make_in_maps(x, freqs_cos, freqs_sin, wq, wk, wv, wo)
    res = run_bass_kernel_spmd(nc, in_maps, list(range(NCORES))).results
    acc = res[0]["out"].astype(np.float64)
    for c in range(1, NCORES):
        acc += res[c]["out"]
    return acc.astype(np.float32)[None]


# revision 13
# speedup vs baseline: 1.0001x; 1.0001x over previous
"""Trainium2 Bass kernel for llama-style GQA attention block.

Problem (hardcoded): x[1,2048,2048] f32, 32 q heads / 8 kv heads, head_dim 64,
RoPE (interleaved pairs), causal mask, out proj. 8-core tensor parallel across
heads: each core owns 4 q heads + 1 kv head, computes its slice end-to-end
including its wo row-block partial product; host sums the 8 partials.

All matmuls run as float32r (fp32 data, fast PE mode, ~1e-4 rel err).
Layout is "feature-on-partition" (transposed) throughout so every matmul
contracts over the partition dim with no on-chip transposes of activations:
  QT = wq^T x^T       (via lhsT=wq, rhs=xT)
  ST = K Q^T          (via lhsT=KT, rhs=QT)   -> softmax along partitions
  OT = [V|1]^T PT     (via lhsT=Vext, rhs=PT) -> row 64 = softmax denominator
  out = OT^T wo       (via lhsT=OT, rhs=wo)
RoPE even/odd pairs are separated by a host-side wq/wk column permutation:
all 4 heads' even-index dims occupy partitions 0..127 of QeP (32 rows per
head), odds in QoP. The kv head's rotated K is replicated 4x along
partitions (KrepE/KrepO) so the 4 heads' score matmuls run as concurrent
32-row PE row-groups (tile_position 0/32/64/96), contracting K=32 each with
even+odd accumulated in PSUM.
"""

import numpy as np

import concourse.bass as bass
import concourse.bacc as bacc
import concourse.mybir as mybir
from concourse.tile import TileContext
from concourse.bass_utils import run_bass_kernel_spmd

DIM = 2048
SEQ = 2048
N_HEADS = 32
N_KV = 8
HD = 64
NCORES = 8
HPC = N_HEADS // NCORES      # 4 q heads per core
SC = 512                     # seq chunk (matmul free dim)
NSC = SEQ // SC              # 4
KTILE = 128
NKT = SEQ // KTILE           # 16
NDCH = DIM // 128            # 16 contraction chunks for projections
F32 = mybir.dt.float32
F32R = mybir.dt.float32r
NEG = -1.0e30

_CACHE = {}


def _build_nc():
    nc = bacc.Bacc("TRN2", debug=False, num_devices=NCORES)
    xT_p = nc.declare_dram_parameter("xT", [DIM, SEQ], F32R, isOutput=False)
    wq_p = nc.declare_dram_parameter("wq", [DIM, HPC * HD], F32R, isOutput=False)
    wkv_p = nc.declare_dram_parameter("wkv", [DIM, 2 * HD], F32R, isOutput=False)
    wo_p = nc.declare_dram_parameter("wo", [HPC * HD, DIM], F32R, isOutput=False)
    cs_p = nc.declare_dram_parameter("cs", [256, SEQ], F32, isOutput=False)
    pat_p = nc.declare_dram_parameter("pat", [4 * KTILE, SC], F32, isOutput=False)
    id_p = nc.declare_dram_parameter("ident", [HD, HD], F32, isOutput=False)
    out_p = nc.declare_dram_parameter("out", [SEQ, DIM], F32, isOutput=True)

    xT_r = xT_p.rearrange("(k p) s -> k p s", p=128)
    wq_r = wq_p.rearrange("(k p) m -> p k m", p=128)
    wkv_r = wkv_p.rearrange("(k p) m -> p k m", p=128)
    pat_r = pat_p.rearrange("(j p) q -> p j q", p=128)
    EXP = mybir.ActivationFunctionType.Exp

    with TileContext(nc) as tc:
        with (
            tc.tile_pool(name="res", bufs=1) as res,
            tc.tile_pool(name="sb", bufs=2) as sb,
            tc.tile_pool(name="psum", bufs=1, space="PSUM") as psum,
        ):
            ps_ctr = [0]

            def ps_tile(idx, shape=(128, SC)):
                ps_ctr[0] += 1
                return psum.tile(list(shape), F32, tag=f"p{idx}", name=f"ps{ps_ctr[0]}")

            # ---- resident loads ----
            wq_t = res.tile([128, NDCH, HPC * HD], F32R, tag="wq_t")
            nc.sync.dma_start(out=wq_t[:], in_=wq_r)
            wkv_t = res.tile([128, NDCH, 2 * HD], F32R, tag="wkv_t")
            nc.sync.dma_start(out=wkv_t[:], in_=wkv_r)
            wo0_t = res.tile([128, DIM], F32R, tag="wo0_t")
            nc.sync.dma_start(out=wo0_t[:], in_=wo_p[0:128, :])
            wo1_t = res.tile([128, DIM], F32R, tag="wo1_t")
            nc.sync.dma_start(out=wo1_t[:], in_=wo_p[128:256, :])
            cos4 = res.tile([128, SEQ], F32, tag="cos4")
            nc.sync.dma_start(out=cos4[:], in_=cs_p[0:128, :])
            sin4 = res.tile([128, SEQ], F32, tag="sin4")
            nc.sync.dma_start(out=sin4[:], in_=cs_p[128:256, :])
            pat_t = res.tile([128, 4, SC], F32, tag="pat_t")
            nc.sync.dma_start(out=pat_t[:], in_=pat_r)
            ident = res.tile([HD, HD], F32, tag="ident")
            nc.sync.dma_start(out=ident[:], in_=id_p[:, :])

            # ---- resident intermediates ----
            QeP = res.tile([128, SEQ], F32R, tag="QeP")    # 4 heads x 32 even rows
            QoP = res.tile([128, SEQ], F32R, tag="QoP")
            KrepE = res.tile([128, SEQ], F32R, tag="KrepE")  # kv head x4 copies
            KrepO = res.tile([128, SEQ], F32R, tag="KrepO")
            VT_sb = res.tile([HD, SEQ], F32, tag="VT_sb")
            OTn0 = res.tile([128, SEQ], F32R, tag="OTn0")   # heads 0,1 norm out^T
            OTn1 = res.tile([128, SEQ], F32R, tag="OTn1")
            ones_col = res.tile([128, 1], F32, tag="ones_col")
            nc.vector.memset(ones_col[:], 1.0)
            ones_f32 = res.tile([1, HD], F32, tag="ones_f32")
            nc.vector.memset(ones_f32[:], 1.0)
            ones_row = res.tile([1, HD], F32R, tag="ones_row")
            nc.vector.tensor_copy(ones_row[:], ones_f32[:])
            vext = []
            for kt in range(NKT):
                vx = res.tile([128, HD + 1], F32R, tag=f"vx{kt}")
                nc.vector.tensor_copy(vx[:, HD : HD + 1], ones_col[:])
                vext.append(vx)

            # ---- phase 1: QKV projections + rope ----
            for sc in range(NSC):
                slc = slice(sc * SC, (sc + 1) * SC)
                qe_ps = ps_tile(0 + sc % 2)   # all-heads evens
                qo_ps = ps_tile(2 + sc % 2)   # all-heads odds
                kv_ps = ps_tile(4 + sc % 2)
                for k in range(NDCH):
                    xt = sb.tile([128, SC], F32R, tag="xt", bufs=3)
                    nc.sync.dma_start(out=xt[:], in_=xT_r[k, :, slc])
                    st, sp = (k == 0), (k == NDCH - 1)
                    nc.tensor.matmul(qe_ps[:], wq_t[:, k, 0:128], xt[:], start=st, stop=sp)
                    nc.tensor.matmul(qo_ps[:], wq_t[:, k, 128:256], xt[:], start=st, stop=sp)
                    nc.tensor.matmul(kv_ps[:], wkv_t[:, k, :], xt[:], start=st, stop=sp)
                # rope Q: full 128-lane ops
                t1 = sb.tile([128, SC], F32, tag="t1", bufs=2)
                t2 = sb.tile([128, SC], F32, tag="t2", bufs=2)
                nc.vector.tensor_mul(t1[:], qe_ps[:], cos4[:, slc])
                nc.vector.tensor_mul(t2[:], qo_ps[:], sin4[:, slc])
                nc.vector.tensor_sub(QeP[:, slc], t1[:], t2[:])
                t3 = sb.tile([128, SC], F32, tag="t3", bufs=2)
                t4 = sb.tile([128, SC], F32, tag="t4", bufs=2)
                nc.vector.tensor_mul(t3[:], qo_ps[:], cos4[:, slc])
                nc.vector.tensor_mul(t4[:], qe_ps[:], sin4[:, slc])
                nc.vector.tensor_add(QoP[:, slc], t3[:], t4[:])
                # rope K (rows 0:32 even / 32:64 odd of kv_ps), then replicate x4
                k1 = sb.tile([32, SC], F32, tag="k1", bufs=2)
                k2 = sb.tile([32, SC], F32, tag="k2", bufs=2)
                nc.vector.tensor_mul(k1[:], kv_ps[0:32, :], cos4[0:32, slc])
                nc.vector.tensor_mul(k2[:], kv_ps[32:64, :], sin4[0:32, slc])
                nc.vector.tensor_sub(KrepE[0:32, slc], k1[:], k2[:])
                k3 = sb.tile([32, SC], F32, tag="k3", bufs=2)
                k4 = sb.tile([32, SC], F32, tag="k4", bufs=2)
                nc.vector.tensor_mul(k3[:], kv_ps[32:64, :], cos4[0:32, slc])
                nc.vector.tensor_mul(k4[:], kv_ps[0:32, :], sin4[0:32, slc])
                nc.vector.tensor_add(KrepO[0:32, slc], k3[:], k4[:])
                for r in (32, 64, 96):
                    nc.vector.tensor_copy(KrepE[r : r + 32, slc], KrepE[0:32, slc])
                    nc.vector.tensor_copy(KrepO[r : r + 32, slc], KrepO[0:32, slc])
                # V passthrough
                nc.any.tensor_copy(VT_sb[:, slc], kv_ps[HD:128, :])

            # ---- phase 1.5: V transpose into [sk, hd | 1] tiles ----
            for kt in range(NKT):
                vt_ps = ps_tile(6 + kt % 2, (128, HD))
                nc.tensor.transpose(vt_ps[:], VT_sb[:, kt * 128 : (kt + 1) * 128], ident[:])
                nc.any.tensor_copy(vext[kt][:, 0:HD], vt_ps[:])

            # ---- phase 2: attention, 4 heads as concurrent PE row groups ----
            for sc in range(NSC):
                slc = slice(sc * SC, (sc + 1) * SC)
                nkt_h = 4 * sc + 4
                o_ps = [ps_tile(4 + h, (HD + 1, SC)) for h in range(HPC)]
                for kt in range(nkt_h):
                    ksl = slice(kt * 128, (kt + 1) * 128)
                    st_ps = [ps_tile(h) for h in range(HPC)]
                    for h in range(HPC):
                        rows = slice(32 * h, 32 * h + 32)
                        tp = (96, 0) if h == 3 else None
                        nc.tensor.matmul(st_ps[h][:], KrepE[rows, ksl], QeP[rows, slc],
                                         start=True, stop=False, tile_position=tp)
                        nc.tensor.matmul(st_ps[h][:], KrepO[rows, ksl], QoP[rows, slc],
                                         start=False, stop=True, tile_position=tp)
                    j = kt - 4 * sc
                    for h in range(HPC):
                        if j >= 0:
                            nc.vector.tensor_add(st_ps[h][:], st_ps[h][:], pat_t[:, j, :])
                        ptile = sb.tile([128, SC], F32R, tag="pt", bufs=6)
                        nc.scalar.activation(ptile[:], st_ps[h][:], EXP, scale=0.125)
                        nc.tensor.matmul(o_ps[h][:], vext[kt][:], ptile[:],
                                         start=(kt == 0), stop=(kt == nkt_h - 1))
                for h in range(HPC):
                    g, hh = h // 2, h % 2
                    recip = sb.tile([1, SC], F32R, tag="recip", bufs=2)
                    with nc.allow_low_precision(reason="f32r is fp32-width"):
                        nc.vector.reciprocal(recip[:], o_ps[h][HD : HD + 1, :])
                    bc_ps = ps_tile(h, (HD, SC))
                    nc.tensor.matmul(bc_ps[:], ones_row[:], recip[:], start=True, stop=True)
                    ou_sb = sb.tile([HD, SC], F32, tag="ou", bufs=2)
                    nc.any.tensor_copy(ou_sb[:], o_ps[h][0:HD, :])
                    dst = (OTn0, OTn1)[g]
                    nc.vector.tensor_mul(dst[64 * hh : 64 * hh + 64, slc], ou_sb[:], bc_ps[:])

            # ---- phase 3: output projection ----
            idx = 0
            for st in range(NKT):
                ssl = slice(st * 128, (st + 1) * 128)
                for dch in range(NSC):
                    dsl = slice(dch * SC, (dch + 1) * SC)
                    op_ps = ps_tile(idx % 4)
                    idx += 1
                    nc.tensor.matmul(op_ps[:], OTn0[:, ssl], wo0_t[:, dsl], start=True, stop=False)
                    nc.tensor.matmul(op_ps[:], OTn1[:, ssl], wo1_t[:, dsl], start=False, stop=True)
                    ot = sb.tile([128, SC], F32, tag="ot", bufs=4)
                    nc.any.tensor_copy(ot[:], op_ps[:])
                    nc.sync.dma_start(out=out_p[ssl, dsl], in_=ot[:])

    nc.compile()
    return nc


def _host_prep(x, freqs_cos, freqs_sin):
    """Shared (core-independent) host-side tensors."""
    xT = np.ascontiguousarray(np.asarray(x, np.float32)[0].T)          # [DIM, SEQ]
    cosT = np.ascontiguousarray(np.asarray(freqs_cos, np.float32).T)   # [32, SEQ]
    sinT = np.ascontiguousarray(np.asarray(freqs_sin, np.float32).T)
    cs = np.concatenate([np.tile(cosT, (4, 1)), np.tile(sinT, (4, 1))], 0)  # [256, SEQ]
    kk = np.arange(4 * KTILE)[:, None]
    qq = np.arange(SC)[None, :]
    pat = np.where(kk <= qq, 0.0, NEG).astype(np.float32)              # [512, 512]
    return xT, cs, pat


def _perm_q():
    """wq columns -> [all heads' even dims (4x32), all heads' odd dims]."""
    ev = [h * HD + 2 * i for h in range(HPC) for i in range(HD // 2)]
    od = [h * HD + 2 * i + 1 for h in range(HPC) for i in range(HD // 2)]
    return ev + od


def _perm_k():
    """wk columns (single head) -> [even dims (32), odd dims (32)]."""
    return [2 * i for i in range(HD // 2)] + [2 * i + 1 for i in range(HD // 2)]


def _is_causal(mask):
    m = np.asarray(mask)
    if m.shape != (SEQ, SEQ):
        return False
    tril = np.tril(np.ones((SEQ, SEQ), bool))
    return bool(np.all(m[tril] == 0.0) and np.all(np.isneginf(m[~tril])))


def _numpy_fallback(x, freqs_cos, freqs_sin, mask, wq, wk, wv, wo):
    x = np.asarray(x, np.float64)
    b, s, _ = x.shape
    xq = (x @ wq).reshape(b, s, N_HEADS, HD)
    xk = (x @ wk).reshape(b, s, N_KV, HD)
    xv = (x @ wv).reshape(b, s, N_KV, HD)

    def rope(t):
        t2 = t.reshape(*t.shape[:-1], HD // 2, 2)
        te, to = t2[..., 0], t2[..., 1]
        c = np.asarray(freqs_cos, np.float64)[None, :, None, :]
        sn = np.asarray(freqs_sin, np.float64)[None, :, None, :]
        oe = te * c - to * sn
        oo = te * sn + to * c
        return np.stack([oe, oo], -1).reshape(t.shape)

    xq, xk = rope(xq), rope(xk)
    xk = np.repeat(xk, N_HEADS // N_KV, axis=2)
    xv = np.repeat(xv, N_HEADS // N_KV, axis=2)
    sc_ = np.einsum("bqhd,bkhd->bhqk", xq, xk) / np.sqrt(HD)
    sc_ = sc_ + np.asarray(mask, np.float64)[None, None]
    m = sc_.max(-1, keepdims=True)
    p = np.exp(sc_ - m)
    p = p / p.sum(-1, keepdims=True)
    out = np.einsum("bhqk,bkhd->bqhd", p, xv).reshape(b, s, N_HEADS * HD)
    return (out @ wo).astype(np.float32)


def _make_in_maps(x, freqs_cos, freqs_sin, wq, wk, wv, wo):
    xT, cs, pat = _host_prep(x, freqs_cos, freqs_sin)
    wq = np.asarray(wq, np.float32)
    wk = np.asarray(wk, np.float32)
    wv = np.asarray(wv, np.float32)
    wo = np.asarray(wo, np.float32)
    permq = _perm_q()
    permk = _perm_k()
    in_maps = []
    for c in range(NCORES):
        wq_c = np.ascontiguousarray(wq[:, c * 256 : (c + 1) * 256][:, permq])
        wk_c = wk[:, c * HD : (c + 1) * HD][:, permk]
        wv_c = wv[:, c * HD : (c + 1) * HD]
        wkv_c = np.ascontiguousarray(np.concatenate([wk_c, wv_c], 1))
        wo_c = np.ascontiguousarray(wo[c * 256 : (c + 1) * 256, :])
        im = {"xT": xT, "wq": wq_c, "wkv": wkv_c, "wo": wo_c, "cs": cs, "pat": pat,
              "ident": np.eye(HD, dtype=np.float32)}
        expect = {"xT": (DIM, SEQ), "wq": (DIM, HPC * HD), "wkv": (DIM, 2 * HD),
                  "wo": (HPC * HD, DIM), "cs": (256, SEQ), "pat": (4 * KTILE, SC),
                  "ident": (HD, HD)}
        for k_, v_ in im.items():
            assert v_.shape == expect[k_], (k_, v_.shape, expect[k_])
        in_maps.append(im)
    return in_maps


def get_nc():
    if "nc" not in _CACHE:
        _CACHE["nc"] = _build_nc()
    return _CACHE["nc"]


def kernel(x, freqs_cos, freqs_sin, mask, wq, wk, wv, wo):
    if not _is_causal(mask):
        return _numpy_fallback(x, freqs_cos, freqs_sin, mask, wq, wk, wv, wo)
    nc = get_nc()
    in_maps = _make_in_maps(x, freqs_cos, freqs_sin, wq, wk, wv, wo)
    res = run_bass_kernel_spmd(nc, in_maps, list(range(NCORES))).results
    acc = res[0]["out"].astype(np.float64)
    for c in range(1, NCORES):
        acc += res[c]["out"]
    return acc.astype(np.float32)[None]


# revision 20
# speedup vs baseline: 4687.4558x; 4687.0010x over previous
"""Trainium2 Bass kernel for llama-style GQA attention block.

Problem (hardcoded): x[1,2048,2048] f32, 32 q heads / 8 kv heads, head_dim 64,
RoPE (interleaved pairs), causal mask, out proj. 8-core tensor parallel across
heads: each core owns 4 q heads + 1 kv head, computes its slice end-to-end
including its wo row-block partial product; host sums the 8 partials.

All matmuls run as float32r (fp32 data, fast PE mode, ~1e-4 rel err).
Layout is "feature-on-partition" (transposed) throughout so every matmul
contracts over the partition dim with no on-chip transposes of activations:
  QT = wq^T x^T       (via lhsT=wq, rhs=xT)
  ST = K Q^T          (via lhsT=KT, rhs=QT)   -> softmax along partitions
  OT = [V|1]^T PT     (via lhsT=Vext, rhs=PT) -> row 64 = softmax denominator
  out = OT^T wo       (via lhsT=OT, rhs=wo)
RoPE even/odd pairs are separated by a host-side wq/wk column permutation:
all 4 heads' even-index dims occupy partitions 0..127 of QeP (32 rows per
head), odds in QoP. The kv head's rotated K is replicated 4x along
partitions (KrepE/KrepO) so the 4 heads' score matmuls run as concurrent
32-row PE row-groups (tile_position 0/32/64/96), contracting K=32 each with
even+odd accumulated in PSUM.
"""

import numpy as np

import concourse.bass as bass
import concourse.bacc as bacc
import concourse.mybir as mybir
from concourse.tile import TileContext
from concourse.bass_utils import run_bass_kernel_spmd

DIM = 2048
SEQ = 2048
N_HEADS = 32
N_KV = 8
HD = 64
NCORES = 8
HPC = N_HEADS // NCORES      # 4 q heads per core
SC = 512                     # seq chunk (matmul free dim)
NSC = SEQ // SC              # 4
KTILE = 128
NKT = SEQ // KTILE           # 16
NDCH = DIM // 128            # 16 contraction chunks for projections
F32 = mybir.dt.float32
F32R = mybir.dt.float32r
NEG = -1.0e30

_CACHE = {}


def _build_nc(reps=1):
    nc = bacc.Bacc("TRN2", debug=False, num_devices=NCORES)
    xT_p = nc.declare_dram_parameter("xT", [DIM, SEQ], F32R, isOutput=False)
    wq_p = nc.declare_dram_parameter("wq", [DIM, HPC * HD], F32R, isOutput=False)
    wkv_p = nc.declare_dram_parameter("wkv", [DIM, 2 * HD], F32R, isOutput=False)
    wo_p = nc.declare_dram_parameter("wo", [HPC * HD, DIM], F32R, isOutput=False)
    cs_p = nc.declare_dram_parameter("cs", [256, SEQ], F32, isOutput=False)
    pat_p = nc.declare_dram_parameter("pat", [KTILE, KTILE], F32, isOutput=False)
    id_p = nc.declare_dram_parameter("ident", [HD, HD], F32, isOutput=False)
    out_p = nc.declare_dram_parameter("out", [SEQ, DIM], F32, isOutput=True)

    xT_r = xT_p.rearrange("(k p) s -> k p s", p=128)
    wq_r = wq_p.rearrange("(k p) m -> p k m", p=128)
    wkv_r = wkv_p.rearrange("(k p) m -> p k m", p=128)
    EXP = mybir.ActivationFunctionType.Exp

    with TileContext(nc) as tc:
        with (
            tc.tile_pool(name="res", bufs=1) as res,
            tc.tile_pool(name="sb", bufs=2) as sb,
            tc.tile_pool(name="psum", bufs=1, space="PSUM") as psum,
        ):
            ps_ctr = [0]

            def ps_tile(idx, shape=(128, SC)):
                ps_ctr[0] += 1
                return psum.tile(list(shape), F32, tag=f"p{idx}", name=f"ps{ps_ctr[0]}")

            # ---- resident allocs ----
            wq_t = res.tile([128, NDCH, HPC * HD], F32R, tag="wq_t")
            wkv_t = res.tile([128, NDCH, 2 * HD], F32R, tag="wkv_t")
            wo0_t = res.tile([128, DIM], F32R, tag="wo0_t")
            wo1_t = res.tile([128, DIM], F32R, tag="wo1_t")
            cos4 = res.tile([128, SEQ], F32, tag="cos4")
            sin4 = res.tile([128, SEQ], F32, tag="sin4")
            pat_t = res.tile([128, 128], F32, tag="pat_t")
            ident = res.tile([HD, HD], F32, tag="ident")

            # ---- resident intermediates ----
            QeP = res.tile([128, SEQ], F32R, tag="QeP")    # 4 heads x 32 even rows
            QoP = res.tile([128, SEQ], F32R, tag="QoP")
            KrepE = res.tile([128, SEQ], F32R, tag="KrepE")  # kv head x4 copies
            KrepO = res.tile([128, SEQ], F32R, tag="KrepO")
            VT_sb = res.tile([HD, SEQ], F32, tag="VT_sb")
            OTn0 = res.tile([128, SEQ], F32R, tag="OTn0")   # heads 0,1 norm out^T
            OTn1 = res.tile([128, SEQ], F32R, tag="OTn1")
            ones_col = res.tile([128, 1], F32, tag="ones_col")
            nc.vector.memset(ones_col[:], 1.0)
            ones_f32 = res.tile([1, HD], F32, tag="ones_f32")
            nc.vector.memset(ones_f32[:], 1.0)
            ones_row = res.tile([1, HD], F32R, tag="ones_row")
            nc.vector.tensor_copy(ones_row[:], ones_f32[:])
            vext = []
            for kt in range(NKT):
                vx = res.tile([128, HD + 1], F32R, tag=f"vx{kt}")
                nc.vector.tensor_copy(vx[:, HD : HD + 1], ones_col[:])
                vext.append(vx)

            for _rep in range(reps):
                for sc in range(NSC):
                    slc = slice(sc * SC, (sc + 1) * SC)
                    # ---- proj(sc): QKV projections (banks p0, p1, p2) ----
                    qe_ps = ps_tile(0)
                    qo_ps = ps_tile(1)
                    kv_ps = ps_tile(2)
                    for k in range(NDCH):
                        # stagger weight/constant loads into chunk 0's k-loop
                        if sc == 0:
                            nc.sync.dma_start(out=wq_t[:, k, :], in_=wq_r[:, k, :])
                            nc.sync.dma_start(out=wkv_t[:, k, :], in_=wkv_r[:, k, :])
                            if k == 1:
                                nc.sync.dma_start(out=cos4[:], in_=cs_p[0:128, :])
                                nc.sync.dma_start(out=sin4[:], in_=cs_p[128:256, :])
                            if k == 2:
                                nc.sync.dma_start(out=pat_t[:], in_=pat_p[:, :])
                                nc.sync.dma_start(out=ident[:], in_=id_p[:, :])
                            if k == 8:
                                nc.sync.dma_start(out=wo0_t[:], in_=wo_p[0:128, :])
                            if k == 12:
                                nc.sync.dma_start(out=wo1_t[:], in_=wo_p[128:256, :])
                        xt = sb.tile([128, SC], F32R, tag="xt", bufs=3)
                        nc.sync.dma_start(out=xt[:], in_=xT_r[k, :, slc])
                        st, sp = (k == 0), (k == NDCH - 1)
                        nc.tensor.matmul(qe_ps[:], wq_t[:, k, 0:128], xt[:], start=st, stop=sp)
                        nc.tensor.matmul(qo_ps[:], wq_t[:, k, 128:256], xt[:], start=st, stop=sp)
                        nc.tensor.matmul(kv_ps[:], wkv_t[:, k, :], xt[:], start=st, stop=sp)
                    # rope Q (full 128-lane)
                    t1 = sb.tile([128, SC], F32, tag="t1", bufs=2)
                    t2 = sb.tile([128, SC], F32, tag="t2", bufs=2)
                    nc.vector.tensor_mul(t1[:], qe_ps[:], cos4[:, slc])
                    nc.vector.tensor_mul(t2[:], qo_ps[:], sin4[:, slc])
                    nc.vector.tensor_sub(QeP[:, slc], t1[:], t2[:])
                    t3 = sb.tile([128, SC], F32, tag="t3", bufs=2)
                    t4 = sb.tile([128, SC], F32, tag="t4", bufs=2)
                    nc.vector.tensor_mul(t3[:], qo_ps[:], cos4[:, slc])
                    nc.vector.tensor_mul(t4[:], qe_ps[:], sin4[:, slc])
                    nc.vector.tensor_add(QoP[:, slc], t3[:], t4[:])
                    # rope K + replicate x4
                    k1 = sb.tile([32, SC], F32, tag="k1", bufs=2)
                    k2 = sb.tile([32, SC], F32, tag="k2", bufs=2)
                    nc.vector.tensor_mul(k1[:], kv_ps[0:32, :], cos4[0:32, slc])
                    nc.vector.tensor_mul(k2[:], kv_ps[32:64, :], sin4[0:32, slc])
                    nc.vector.tensor_sub(KrepE[0:32, slc], k1[:], k2[:])
                    k3 = sb.tile([32, SC], F32, tag="k3", bufs=2)
                    k4 = sb.tile([32, SC], F32, tag="k4", bufs=2)
                    nc.vector.tensor_mul(k3[:], kv_ps[32:64, :], cos4[0:32, slc])
                    nc.vector.tensor_mul(k4[:], kv_ps[0:32, :], sin4[0:32, slc])
                    nc.vector.tensor_add(KrepO[0:32, slc], k3[:], k4[:])
                    for r in (32, 64, 96):
                        nc.vector.tensor_copy(KrepE[r : r + 32, slc], KrepE[0:32, slc])
                        nc.vector.tensor_copy(KrepO[r : r + 32, slc], KrepO[0:32, slc])
                    # V passthrough
                    nc.any.tensor_copy(VT_sb[:, slc], kv_ps[HD:128, :])

                    # ---- vtrans(sc): V transpose for this chunk (bank p7) ----
                    for kt in range(4 * sc, 4 * sc + 4):
                        vt_ps = ps_tile(7, (128, HD))
                        nc.tensor.transpose(vt_ps[:], VT_sb[:, kt * 128 : (kt + 1) * 128], ident[:])
                        nc.any.tensor_copy(vext[kt][:, 0:HD], vt_ps[:])

                    # ---- attention(sc): head pairs, banks st p3/p4 ot p5/p6 ----
                    nkt_h = 4 * sc + 4
                    for hp in range(2):
                        heads = (2 * hp, 2 * hp + 1)
                        o_ps = {h: ps_tile(5 + i, (HD + 1, SC)) for i, h in enumerate(heads)}
                        for kt in range(nkt_h):
                            ksl = slice(kt * 128, (kt + 1) * 128)
                            j = kt - 4 * sc
                            # diagonal tiles: only q >= k-tile start contributes
                            qo = 128 * j if j > 0 else 0      # q offset within chunk
                            nv = SC - qo                      # valid q count
                            qsl = slice(sc * SC + qo, (sc + 1) * SC)
                            st_ps = {h: ps_tile(3 + i) for i, h in enumerate(heads)}
                            for h in heads:
                                rows = slice(32 * h, 32 * h + 32)
                                tp = (96, 0) if h == 3 else None
                                nc.tensor.matmul(st_ps[h][:, 0:nv], KrepE[rows, ksl], QeP[rows, qsl],
                                                 start=True, stop=False, tile_position=tp)
                                nc.tensor.matmul(st_ps[h][:, 0:nv], KrepO[rows, ksl], QoP[rows, qsl],
                                                 start=False, stop=True, tile_position=tp)
                            for h in heads:
                                if j >= 0:
                                    # triangle mask on the first 128 valid columns
                                    nc.vector.tensor_add(st_ps[h][:, 0:128], st_ps[h][:, 0:128], pat_t[:])
                                ptile = sb.tile([128, SC], F32R, tag="pt", bufs=6)
                                nc.scalar.activation(ptile[:, 0:nv], st_ps[h][:, 0:nv], EXP, scale=0.125)
                                nc.tensor.matmul(o_ps[h][:, qo : qo + nv], vext[kt][:], ptile[:, 0:nv],
                                                 start=(kt == 0), stop=(kt == nkt_h - 1))
                        for h in heads:
                            g, hh = h // 2, h % 2
                            recip = sb.tile([1, SC], F32R, tag="recip", bufs=2)
                            with nc.allow_low_precision(reason="f32r is fp32-width"):
                                nc.vector.reciprocal(recip[:], o_ps[h][HD : HD + 1, :])
                            bc_ps = ps_tile(3 + (h % 2), (HD, SC))
                            nc.tensor.matmul(bc_ps[:], ones_row[:], recip[:], start=True, stop=True)
                            ou_sb = sb.tile([HD, SC], F32, tag="ou", bufs=2)
                            nc.any.tensor_copy(ou_sb[:], o_ps[h][0:HD, :])
                            dst = (OTn0, OTn1)[g]
                            nc.vector.tensor_mul(dst[64 * hh : 64 * hh + 64, slc], ou_sb[:], bc_ps[:])

                    # ---- outproj(sc): rows of this chunk (bank p7) ----
                    for st in range(4 * sc, 4 * sc + 4):
                        ssl = slice(st * 128, (st + 1) * 128)
                        for dch in range(NSC):
                            dsl = slice(dch * SC, (dch + 1) * SC)
                            op_ps = ps_tile(7)
                            nc.tensor.matmul(op_ps[:], OTn0[:, ssl], wo0_t[:, dsl], start=True, stop=False)
                            nc.tensor.matmul(op_ps[:], OTn1[:, ssl], wo1_t[:, dsl], start=False, stop=True)
                            ot = sb.tile([128, SC], F32, tag="ot", bufs=4)
                            if (st + dch) % 2 == 0:
                                nc.vector.tensor_copy(ot[:], op_ps[:])
                            else:
                                nc.scalar.copy(ot[:], op_ps[:])
                            nc.sync.dma_start(out=out_p[ssl, dsl], in_=ot[:])

    nc.compile()
    return nc


def _host_prep(x, freqs_cos, freqs_sin):
    """Shared (core-independent) host-side tensors."""
    xT = np.ascontiguousarray(np.asarray(x, np.float32)[0].T)          # [DIM, SEQ]
    cosT = np.ascontiguousarray(np.asarray(freqs_cos, np.float32).T)   # [32, SEQ]
    sinT = np.ascontiguousarray(np.asarray(freqs_sin, np.float32).T)
    cs = np.concatenate([np.tile(cosT, (4, 1)), np.tile(sinT, (4, 1))], 0)  # [256, SEQ]
    kk = np.arange(KTILE)[:, None]
    qq = np.arange(KTILE)[None, :]
    pat = np.where(kk <= qq, 0.0, NEG).astype(np.float32)              # [128, 128]
    return xT, cs, pat


def _perm_q():
    """wq columns -> [all heads' even dims (4x32), all heads' odd dims]."""
    ev = [h * HD + 2 * i for h in range(HPC) for i in range(HD // 2)]
    od = [h * HD + 2 * i + 1 for h in range(HPC) for i in range(HD // 2)]
    return ev + od


def _perm_k():
    """wk columns (single head) -> [even dims (32), odd dims (32)]."""
    return [2 * i for i in range(HD // 2)] + [2 * i + 1 for i in range(HD // 2)]


def _is_causal(mask):
    m = np.asarray(mask)
    if m.shape != (SEQ, SEQ):
        return False
    tril = np.tril(np.ones((SEQ, SEQ), bool))
    return bool(np.all(m[tril] == 0.0) and np.all(np.isneginf(m[~tril])))


def _numpy_fallback(x, freqs_cos, freqs_sin, mask, wq, wk, wv, wo):
    x = np.asarray(x, np.float64)
    b, s, _ = x.shape
    xq = (x @ wq).reshape(b, s, N_HEADS, HD)
    xk = (x @ wk).reshape(b, s, N_KV, HD)
    xv = (x @ wv).reshape(b, s, N_KV, HD)

    def rope(t):
        t2 = t.reshape(*t.shape[:-1], HD // 2, 2)
        te, to = t2[..., 0], t2[..., 1]
        c = np.asarray(freqs_cos, np.float64)[None, :, None, :]
        sn = np.asarray(freqs_sin, np.float64)[None, :, None, :]
        oe = te * c - to * sn
        oo = te * sn + to * c
        return np.stack([oe, oo], -1).reshape(t.shape)

    xq, xk = rope(xq), rope(xk)
    xk = np.repeat(xk, N_HEADS // N_KV, axis=2)
    xv = np.repeat(xv, N_HEADS // N_KV, axis=2)
    sc_ = np.einsum("bqhd,bkhd->bhqk", xq, xk) / np.sqrt(HD)
    sc_ = sc_ + np.asarray(mask, np.float64)[None, None]
    m = sc_.max(-1, keepdims=True)
    p = np.exp(sc_ - m)
    p = p / p.sum(-1, keepdims=True)
    out = np.einsum("bhqk,bkhd->bqhd", p, xv).reshape(b, s, N_HEADS * HD)
    return (out @ wo).astype(np.float32)


def _make_in_maps(x, freqs_cos, freqs_sin, wq, wk, wv, wo):
    xT, cs, pat = _host_prep(x, freqs_cos, freqs_sin)
    wq = np.asarray(wq, np.float32)
    wk = np.asarray(wk, np.float32)
    wv = np.asarray(wv, np.float32)
    wo = np.asarray(wo, np.float32)
    permq = _perm_q()
    permk = _perm_k()
    in_maps = []
    for c in range(NCORES):
        wq_c = np.ascontiguousarray(wq[:, c * 256 : (c + 1) * 256][:, permq])
        wk_c = wk[:, c * HD : (c + 1) * HD][:, permk]
        wv_c = wv[:, c * HD : (c + 1) * HD]
        wkv_c = np.ascontiguousarray(np.concatenate([wk_c, wv_c], 1))
        wo_c = np.ascontiguousarray(wo[c * 256 : (c + 1) * 256, :])
        im = {"xT": xT, "wq": wq_c, "wkv": wkv_c, "wo": wo_c, "cs": cs, "pat": pat,
              "ident": np.eye(HD, dtype=np.float32)}
        expect = {"xT": (DIM, SEQ), "wq": (DIM, HPC * HD), "wkv": (DIM, 2 * HD),
                  "wo": (HPC * HD, DIM), "cs": (256, SEQ), "pat": (KTILE, KTILE),
                  "ident": (HD, HD)}
        for k_, v_ in im.items():
            assert v_.shape == expect[k_], (k_, v_.shape, expect[k_])
        in_maps.append(im)
    return in_maps


def get_nc():
    if "nc" not in _CACHE:
        _CACHE["nc"] = _build_nc()
    return _CACHE["nc"]


def kernel(x, freqs_cos, freqs_sin, mask, wq, wk, wv, wo):
    if not _is_causal(mask):
        return _numpy_fallback(x, freqs_cos, freqs_sin, mask, wq, wk, wv, wo)
    nc = get_nc()
    in_maps = _make_in_maps(x, freqs_cos, freqs_sin, wq, wk, wv, wo)
    res = run_bass_kernel_spmd(nc, in_maps, list(range(NCORES))).results
    acc = res[0]["out"].astype(np.float64)
    for c in range(1, NCORES):
        acc += res[c]["out"]
    return acc.astype(np.float32)[None]
